# revision 15
# baseline (speedup 1.0000x reference)
"""Biaffine kernel for Trainium2, data-parallel over batch across 8 NeuronCores.

Reference math (per batch b):
    Daug = [D, 1]                                  # [S, d+1]
    out  = Daug @ U @ H^T + (Daug @ W[:d+1])[:, None] + (H @ W[d+1:])[None, :]

Algebraic refactor used here (d = 1024):
    U0 = U[:d]                # [d, d]
    c  = U[d] + W[d+1:]      # [d]     (folds the ones-row of Daug and the H-linear term)
    T' = D @ U0 + c           # [S, d]
    dlin = D @ W[:d] + W[d]   # [S]     (tiny; computed host-side)
    out  = T' @ H^T + dlin[:, None]

Device kernel per core (4 batches):
    matmul1: T'^T[j, x] = sum_k U0[k, j] * D^T[k, x]   (lhsT = U0 natural, rhs = D^T)
             + per-partition bias c[j] fused into PSUM->SBUF copy (DVE)
    matmul2: out[x, y]  = sum_j T'^T[j, x] * H^T[j, y] (lhsT = T'^T, rhs = H^T)
             + per-partition bias dlin[x] fused into PSUM->SBUF copy (DVE)

D^T / H^T are produced host-side (numpy transpose) so the device does zero
transposes and the PE runs a dense stream of 384 matmuls per core.

Matmul dtype: float32r (fp32 with 11-bit mantissa, single-pass PE at
1 cycle/row) by default; BIAFFINE_MM_F32=1 switches to exact fp32
(2 half-rate passes, ~4x slower PE). Accumulation is fp32 PSUM either way.
fp32r operands must be pre-rounded (low 12 mantissa bits zeroed, RNE);
for the DRAM inputs that is done host-side for free.
"""

import os
import sys

import numpy as np

for _p in (
    "/root/.axon_site",
    "/root/.axon_site/_ro/trn_rl_repo",
    "/root/.axon_site/_ro/pypackages",
    "/opt/trn_rl_repo",
):
    if os.path.isdir(_p) and _p not in sys.path:
        sys.path.append(_p)

import concourse.bass as bass
import concourse.mybir as mybir
import concourse.tile as tile
from concourse import bacc
from concourse.bass_utils import run_bass_kernel_spmd

B, S, D_DIM = 32, 512, 1024
N_CORES = 8
BPC = B // N_CORES  # batches per core
KT = D_DIM // 128  # 8 k-tiles (contraction over d)
JT = D_DIM // 128  # 8 j-tiles (M dim of matmul1)
XT = S // 128  # 4 x-tiles (M dim of matmul2)

_NC_CACHE = {}


def _use_f32r() -> bool:
    return os.environ.get("BIAFFINE_MM_F32", "0") != "1"


def _build_nc() -> bass.Bass:
    nc = bacc.Bacc()
    f32 = mybir.dt.float32
    mm_dt = mybir.dt.float32r if _use_f32r() else f32

    # Inputs arrive pre-swizzled to the SBUF layout: [.., p, kt, x] so each
    # partition's DMA read is one contiguous block.
    dt_in = nc.dram_tensor("dt_in", [BPC, 128, KT, S], mm_dt, kind="ExternalInput")
    ht_in = nc.dram_tensor("ht_in", [BPC, 128, KT, S], mm_dt, kind="ExternalInput")
    u0_in = nc.dram_tensor("u0_in", [128, KT, D_DIM], mm_dt, kind="ExternalInput")
    ccol_in = nc.dram_tensor("ccol_in", [128, JT], f32, kind="ExternalInput")
    dcol_in = nc.dram_tensor("dcol_in", [128, BPC * XT], f32, kind="ExternalInput")
    out_t = nc.dram_tensor("out", [BPC, S, S], f32, kind="ExternalOutput")

    with tile.TileContext(nc) as tc:
        with (
            tc.tile_pool(name="const", bufs=1) as cpool,
            tc.tile_pool(name="dh", bufs=3) as dh_pool,
            tc.tile_pool(name="tt", bufs=2) as tt_pool,
            tc.tile_pool(name="ot", bufs=3) as ot_pool,
            tc.tile_pool(name="ps", bufs=8, space="PSUM") as ps_pool,
        ):
            ccol = cpool.tile([128, JT], f32)
            nc.sync.dma_start(ccol[:], ccol_in[:])
            dcol = cpool.tile([128, BPC * XT], f32)
            nc.scalar.dma_start(dcol[:], dcol_in[:])

            # U0 as 8 per-kt tiles; batch-0 D^T/H^T as 8 per-kt chunks.
            # Chunks are issued in consumption order across both HWDGE rings
            # so the first matmul starts as soon as u0[0]+dt[0] land, and
            # batch 0's kt-outer matmul1 consumes chunks at DMA arrival rate.
            u0_t = [cpool.tile([128, D_DIM], mm_dt, tag=f"u0k{kt}", name=f"u0k{kt}") for kt in range(KT)]
            dt_t = [dh_pool.tile([128, S], mm_dt, tag=f"dt{kt}", name=f"dt{kt}") for kt in range(KT)]
            ht_t = [dh_pool.tile([128, S], mm_dt, tag=f"ht{kt}", name=f"ht{kt}") for kt in range(KT)]
            for kt in range(KT):
                if kt == 0:
                    nc.sync.dma_start(u0_t[0][:, :512], u0_in[:, 0, :512])
                    nc.scalar.dma_start(dt_t[0][:, :256], dt_in[0, :, 0, :256])
                    nc.sync.dma_start(u0_t[0][:, 512:], u0_in[:, 0, 512:])
                    nc.scalar.dma_start(dt_t[0][:, 256:], dt_in[0, :, 0, 256:])
                else:
                    nc.sync.dma_start(u0_t[kt][:], u0_in[:, kt, :])
                    nc.scalar.dma_start(dt_t[kt][:], dt_in[0, :, kt, :])
            for kt in range(KT):
                (nc.sync if kt % 2 == 0 else nc.scalar).dma_start(
                    ht_t[kt][:], ht_in[0, :, kt, :]
                )

            for b in range(BPC):
                if b > 0:
                    dt_t = [
                        dh_pool.tile([128, S], mm_dt, tag=f"dt{kt}", name=f"dt{kt}") for kt in range(KT)
                    ]
                    ht_t = [
                        dh_pool.tile([128, S], mm_dt, tag=f"ht{kt}", name=f"ht{kt}") for kt in range(KT)
                    ]
                    for kt in range(KT):
                        nc.sync.dma_start(dt_t[kt][:], dt_in[b, :, kt, :])
                        nc.scalar.dma_start(ht_t[kt][:], ht_in[b, :, kt, :])

                # matmul1: T'^T[jm*128+p, x]  (+ bias c)
                tt_t = [
                    tt_pool.tile([128, S], mm_dt, tag=f"tt{jm}", name=f"tt{jm}") for jm in range(JT)
                ]
                if b == 0:
                    # kt-outer: 8 live PSUM banks; each kt step needs only
                    # chunk kt of u0/dt, so the PE tracks DMA arrivals.
                    ps_l = [
                        ps_pool.tile([128, S], f32, tag="ps", name=f"ps{jm}")
                        for jm in range(JT)
                    ]
                    for kt in range(KT):
                        for jm in range(JT):
                            nc.tensor.matmul(
                                ps_l[jm][:],
                                lhsT=u0_t[kt][:, jm * 128 : (jm + 1) * 128],
                                rhs=dt_t[kt][:],
                                start=(kt == 0),
                                stop=(kt == KT - 1),
                            )
                    for jm in range(JT):
                        nc.vector.tensor_scalar_add(
                            tt_t[jm][:], ps_l[jm][:], ccol[:, jm : jm + 1]
                        )
                else:
                    for jm in range(JT):
                        ps = ps_pool.tile([128, S], f32, tag="ps", name="ps")
                        for kt in range(KT):
                            nc.tensor.matmul(
                                ps[:],
                                lhsT=u0_t[kt][:, jm * 128 : (jm + 1) * 128],
                                rhs=dt_t[kt][:],
                                start=(kt == 0),
                                stop=(kt == KT - 1),
                            )
                        nc.vector.tensor_scalar_add(
                            tt_t[jm][:], ps[:], ccol[:, jm : jm + 1]
                        )

                # matmul2: out[xt*128+p, y]  (+ bias dlin)
                for xt in range(XT):
                    po = ps_pool.tile([128, S], f32, tag="ps", name="po")
                    for jm in range(JT):
                        nc.tensor.matmul(
                            po[:],
                            lhsT=tt_t[jm][:, xt * 128 : (xt + 1) * 128],
                            rhs=ht_t[jm][:],
                            start=(jm == 0),
                            stop=(jm == JT - 1),
                        )
                    ot = ot_pool.tile([128, S], f32, tag="ot", name="ot")
                    nc.vector.tensor_scalar_add(
                        ot[:], po[:], dcol[:, b * XT + xt : b * XT + xt + 1]
                    )
                    st_eng = nc.sync if (b * XT + xt) % 2 == 0 else nc.scalar
                    if b == BPC - 1 and xt == XT - 1:
                        nc.sync.dma_start(out_t[b, xt * 128 : (xt + 1) * 128, :256], ot[:, :256])
                        nc.scalar.dma_start(out_t[b, xt * 128 : (xt + 1) * 128, 256:], ot[:, 256:])
                    else:
                        st_eng.dma_start(out_t[b, xt * 128 : (xt + 1) * 128, :], ot[:])
    nc.finalize()
    return nc


def _get_nc() -> bass.Bass:
    key = "nc_f32r" if _use_f32r() else "nc_f32"
    if key not in _NC_CACHE:
        _NC_CACHE[key] = _build_nc()
    return _NC_CACHE[key]


def _round_fp32r(a: np.ndarray) -> np.ndarray:
    """Round fp32 to fp32r layout: RNE to 11-bit mantissa, low 12 bits zero."""
    bits = np.ascontiguousarray(a, dtype=np.float32).view(np.uint32)
    odd = (bits >> 12) & np.uint32(1)
    out = (bits + np.uint32(0x7FF) + odd) & np.uint32(0xFFFFF000)
    return out.view(np.float32)


def kernel(D, H, U, W):
    D = np.ascontiguousarray(np.asarray(D, dtype=np.float32))
    H = np.ascontiguousarray(np.asarray(H, dtype=np.float32))
    U = np.asarray(U, dtype=np.float32)
    W = np.asarray(W, dtype=np.float32)
    d = D_DIM
    f32r = _use_f32r()

    # U0 swizzled to [128, KT, d]: [p, kt, j] = U0[kt*128+p, j]
    U0 = np.ascontiguousarray(U[:d, :].reshape(KT, 128, d).transpose(1, 0, 2))
    if f32r:
        U0 = _round_fp32r(U0)
    c = (U[d, :] + W[d + 1 :]).astype(np.float32)  # [d]
    # ccol[p, jm] = c[jm*128 + p]
    ccol = np.ascontiguousarray(c.reshape(JT, 128).T)
    # dlin[b, x] = D[b, x] . W[:d] + W[d]
    dlin = (D @ W[:d] + W[d]).astype(np.float32)  # [B, S]

    in_maps = []
    for cidx in range(N_CORES):
        sl = slice(cidx * BPC, (cidx + 1) * BPC)
        # [b, p, kt, x] = X[b, x, kt*128+p]  (transpose + swizzle in one copy)
        Dt = np.ascontiguousarray(D[sl].reshape(BPC, S, KT, 128).transpose(0, 3, 2, 1))
        Ht = np.ascontiguousarray(H[sl].reshape(BPC, S, KT, 128).transpose(0, 3, 2, 1))
        if f32r:
            Dt = _round_fp32r(Dt)
            Ht = _round_fp32r(Ht)
        # dcol[p, b*XT + xt] = dlin[b, xt*128 + p]
        dcol = np.ascontiguousarray(
            dlin[sl].reshape(BPC, XT, 128).transpose(2, 0, 1).reshape(128, BPC * XT)
        )
        in_maps.append(
            {
                "dt_in": Dt,
                "ht_in": Ht,
                "u0_in": U0,
                "ccol_in": ccol,
                "dcol_in": dcol,
            }
        )

    nc = _get_nc()
    trace = bool(int(os.environ.get("BIAFFINE_TRACE", "0")))
    kwargs = {}
    if trace:
        tdir = os.environ.get("BIAFFINE_TRACE_DIR")
        if tdir:
            os.makedirs(tdir, exist_ok=True)
            kwargs["tmpdir"] = tdir
    res = run_bass_kernel_spmd(
        nc, in_maps, core_ids=list(range(N_CORES)), trace=trace, **kwargs
    )
    if trace and res.exec_time_ns is not None:
        print(f"HW exec time: {res.exec_time_ns} ns")

    out = np.concatenate([res.results[i]["out"] for i in range(N_CORES)], axis=0)
    return out


# revision 16
# speedup vs baseline: 1.0091x; 1.0091x over previous
"""Biaffine kernel for Trainium2, data-parallel over batch across 8 NeuronCores.

Reference math (per batch b):
    Daug = [D, 1]                                  # [S, d+1]
    out  = Daug @ U @ H^T + (Daug @ W[:d+1])[:, None] + (H @ W[d+1:])[None, :]

Algebraic refactor used here (d = 1024):
    U0 = U[:d]                # [d, d]
    c  = U[d] + W[d+1:]      # [d]     (folds the ones-row of Daug and the H-linear term)
    T' = D @ U0 + c           # [S, d]
    dlin = D @ W[:d] + W[d]   # [S]     (tiny; computed host-side)
    out  = T' @ H^T + dlin[:, None]

Device kernel per core (4 batches):
    matmul1: T'^T[j, x] = sum_k U0[k, j] * D^T[k, x]   (lhsT = U0 natural, rhs = D^T)
             + per-partition bias c[j] fused into PSUM->SBUF copy (DVE)
    matmul2: out[x, y]  = sum_j T'^T[j, x] * H^T[j, y] (lhsT = T'^T, rhs = H^T)
             + per-partition bias dlin[x] fused into PSUM->SBUF copy (DVE)

D^T / H^T are produced host-side (numpy transpose) so the device does zero
transposes and the PE runs a dense stream of 384 matmuls per core.

Matmul dtype: float32r (fp32 with 11-bit mantissa, single-pass PE at
1 cycle/row) by default; BIAFFINE_MM_F32=1 switches to exact fp32
(2 half-rate passes, ~4x slower PE). Accumulation is fp32 PSUM either way.
fp32r operands must be pre-rounded (low 12 mantissa bits zeroed, RNE);
for the DRAM inputs that is done host-side for free.
"""

import os
import sys

import numpy as np

for _p in (
    "/root/.axon_site",
    "/root/.axon_site/_ro/trn_rl_repo",
    "/root/.axon_site/_ro/pypackages",
    "/opt/trn_rl_repo",
):
    if os.path.isdir(_p) and _p not in sys.path:
        sys.path.append(_p)

import concourse.bass as bass
import concourse.mybir as mybir
import concourse.tile as tile
from concourse import bacc
from concourse.bass_utils import run_bass_kernel_spmd

B, S, D_DIM = 32, 512, 1024
N_CORES = 8
BPC = B // N_CORES  # batches per core
KT = D_DIM // 128  # 8 k-tiles (contraction over d)
JT = D_DIM // 128  # 8 j-tiles (M dim of matmul1)
XT = S // 128  # 4 x-tiles (M dim of matmul2)

_NC_CACHE = {}


def _use_f32r() -> bool:
    return os.environ.get("BIAFFINE_MM_F32", "0") != "1"


def _build_nc() -> bass.Bass:
    nc = bacc.Bacc()
    f32 = mybir.dt.float32
    mm_dt = mybir.dt.float32r if _use_f32r() else f32

    # Inputs arrive pre-swizzled to the SBUF layout: [.., p, kt, x] so each
    # partition's DMA read is one contiguous block.
    dt_in = nc.dram_tensor("dt_in", [BPC, 128, KT, S], mm_dt, kind="ExternalInput")
    ht_in = nc.dram_tensor("ht_in", [BPC, 128, KT, S], mm_dt, kind="ExternalInput")
    u0_in = nc.dram_tensor("u0_in", [128, KT, D_DIM], mm_dt, kind="ExternalInput")
    ccol_in = nc.dram_tensor("ccol_in", [128, JT], f32, kind="ExternalInput")
    dcol_in = nc.dram_tensor("dcol_in", [128, BPC * XT], f32, kind="ExternalInput")
    out_t = nc.dram_tensor("out", [BPC, S, S], f32, kind="ExternalOutput")

    with tile.TileContext(nc) as tc:
        with (
            tc.tile_pool(name="const", bufs=1) as cpool,
            tc.tile_pool(name="dh", bufs=2) as dh_pool,
            tc.tile_pool(name="tt", bufs=2) as tt_pool,
            tc.tile_pool(name="ot", bufs=3) as ot_pool,
            tc.tile_pool(name="ps", bufs=8, space="PSUM") as ps_pool,
        ):
            ccol = cpool.tile([128, JT], f32)
            nc.sync.dma_start(ccol[:], ccol_in[:])
            dcol = cpool.tile([128, BPC * XT], f32)
            nc.scalar.dma_start(dcol[:], dcol_in[:])

            # U0 as 8 per-kt tiles; batch-0 D^T/H^T as 8 per-kt chunks.
            # Chunks are issued in consumption order across both HWDGE rings
            # so the first matmul starts as soon as u0[0]+dt[0] land, and
            # batch 0's kt-outer matmul1 consumes chunks at DMA arrival rate.
            u0_t = [cpool.tile([128, D_DIM], mm_dt, tag=f"u0k{kt}", name=f"u0k{kt}") for kt in range(KT)]
            dt_t = [dh_pool.tile([128, S], mm_dt, tag=f"dt{kt}", name=f"dt{kt}") for kt in range(KT)]
            ht_t = [dh_pool.tile([128, S], mm_dt, tag=f"ht{kt}", name=f"ht{kt}") for kt in range(KT)]
            for kt in range(KT):
                nc.sync.dma_start(u0_t[kt][:], u0_in[:, kt, :])
                nc.scalar.dma_start(dt_t[kt][:], dt_in[0, :, kt, :])
            for kt in range(KT):
                (nc.sync if kt % 2 == 0 else nc.scalar).dma_start(
                    ht_t[kt][:], ht_in[0, :, kt, :]
                )

            for b in range(BPC):
                if b > 0:
                    dt_t = [
                        dh_pool.tile([128, S], mm_dt, tag=f"dt{kt}", name=f"dt{kt}") for kt in range(KT)
                    ]
                    ht_t = [
                        dh_pool.tile([128, S], mm_dt, tag=f"ht{kt}", name=f"ht{kt}") for kt in range(KT)
                    ]
                    for kt in range(KT):
                        nc.sync.dma_start(dt_t[kt][:], dt_in[b, :, kt, :])
                        nc.scalar.dma_start(ht_t[kt][:], ht_in[b, :, kt, :])

                # matmul1: T'^T[jm*128+p, x]  (+ bias c)
                tt_t = [
                    tt_pool.tile([128, S], mm_dt, tag=f"tt{jm}", name=f"tt{jm}") for jm in range(JT)
                ]
                if b == 0:
                    # kt-outer: 8 live PSUM banks; each kt step needs only
                    # chunk kt of u0/dt, so the PE tracks DMA arrivals.
                    ps_l = [
                        ps_pool.tile([128, S], f32, tag="ps", name=f"ps{jm}")
                        for jm in range(JT)
                    ]
                    for kt in range(KT):
                        for jm in range(JT):
                            nc.tensor.matmul(
                                ps_l[jm][:],
                                lhsT=u0_t[kt][:, jm * 128 : (jm + 1) * 128],
                                rhs=dt_t[kt][:],
                                start=(kt == 0),
                                stop=(kt == KT - 1),
                            )
                    for jm in range(JT):
                        nc.vector.tensor_scalar_add(
                            tt_t[jm][:], ps_l[jm][:], ccol[:, jm : jm + 1]
                        )
                else:
                    for jm in range(JT):
                        ps = ps_pool.tile([128, S], f32, tag="ps", name="ps")
                        for kt in range(KT):
                            nc.tensor.matmul(
                                ps[:],
                                lhsT=u0_t[kt][:, jm * 128 : (jm + 1) * 128],
                                rhs=dt_t[kt][:],
                                start=(kt == 0),
                                stop=(kt == KT - 1),
                            )
                        nc.vector.tensor_scalar_add(
                            tt_t[jm][:], ps[:], ccol[:, jm : jm + 1]
                        )

                # matmul2: out[xt*128+p, y]  (+ bias dlin)
                for xt in range(XT):
                    po = ps_pool.tile([128, S], f32, tag="ps", name="po")
                    for jm in range(JT):
                        nc.tensor.matmul(
                            po[:],
                            lhsT=tt_t[jm][:, xt * 128 : (xt + 1) * 128],
                            rhs=ht_t[jm][:],
                            start=(jm == 0),
                            stop=(jm == JT - 1),
                        )
                    ot = ot_pool.tile([128, S], f32, tag="ot", name="ot")
                    nc.vector.tensor_scalar_add(
                        ot[:], po[:], dcol[:, b * XT + xt : b * XT + xt + 1]
                    )
                    if b == BPC - 1 and xt == XT - 1:
                        nc.sync.dma_start(
                            out_t[b, xt * 128 : (xt + 1) * 128, :256], ot[:, :256]
                        )
                        nc.scalar.dma_start(
                            out_t[b, xt * 128 : (xt + 1) * 128, 256:], ot[:, 256:]
                        )
                    else:
                        nc.scalar.dma_start(
                            out_t[b, xt * 128 : (xt + 1) * 128, :], ot[:]
                        )
    nc.finalize()
    return nc


def _get_nc() -> bass.Bass:
    key = "nc_f32r" if _use_f32r() else "nc_f32"
    if key not in _NC_CACHE:
        _NC_CACHE[key] = _build_nc()
    return _NC_CACHE[key]


def _round_fp32r(a: np.ndarray) -> np.ndarray:
    """Round fp32 to fp32r layout: RNE to 11-bit mantissa, low 12 bits zero."""
    bits = np.ascontiguousarray(a, dtype=np.float32).view(np.uint32)
    odd = (bits >> 12) & np.uint32(1)
    out = (bits + np.uint32(0x7FF) + odd) & np.uint32(0xFFFFF000)
    return out.view(np.float32)


def kernel(D, H, U, W):
    D = np.ascontiguousarray(np.asarray(D, dtype=np.float32))
    H = np.ascontiguousarray(np.asarray(H, dtype=np.float32))
    U = np.asarray(U, dtype=np.float32)
    W = np.asarray(W, dtype=np.float32)
    d = D_DIM
    f32r = _use_f32r()

    # U0 swizzled to [128, KT, d]: [p, kt, j] = U0[kt*128+p, j]
    U0 = np.ascontiguousarray(U[:d, :].reshape(KT, 128, d).transpose(1, 0, 2))
    if f32r:
        U0 = _round_fp32r(U0)
    c = (U[d, :] + W[d + 1 :]).astype(np.float32)  # [d]
    # ccol[p, jm] = c[jm*128 + p]
    ccol = np.ascontiguousarray(c.reshape(JT, 128).T)
    # dlin[b, x] = D[b, x] . W[:d] + W[d]
    dlin = (D @ W[:d] + W[d]).astype(np.float32)  # [B, S]

    in_maps = []
    for cidx in range(N_CORES):
        sl = slice(cidx * BPC, (cidx + 1) * BPC)
        # [b, p, kt, x] = X[b, x, kt*128+p]  (transpose + swizzle in one copy)
        Dt = np.ascontiguousarray(D[sl].reshape(BPC, S, KT, 128).transpose(0, 3, 2, 1))
        Ht = np.ascontiguousarray(H[sl].reshape(BPC, S, KT, 128).transpose(0, 3, 2, 1))
        if f32r:
            Dt = _round_fp32r(Dt)
            Ht = _round_fp32r(Ht)
        # dcol[p, b*XT + xt] = dlin[b, xt*128 + p]
        dcol = np.ascontiguousarray(
            dlin[sl].reshape(BPC, XT, 128).transpose(2, 0, 1).reshape(128, BPC * XT)
        )
        in_maps.append(
            {
                "dt_in": Dt,
                "ht_in": Ht,
                "u0_in": U0,
                "ccol_in": ccol,
                "dcol_in": dcol,
            }
        )

    nc = _get_nc()
    trace = bool(int(os.environ.get("BIAFFINE_TRACE", "0")))
    kwargs = {}
    if trace:
        tdir = os.environ.get("BIAFFINE_TRACE_DIR")
        if tdir:
            os.makedirs(tdir, exist_ok=True)
            kwargs["tmpdir"] = tdir
    res = run_bass_kernel_spmd(
        nc, in_maps, core_ids=list(range(N_CORES)), trace=trace, **kwargs
    )
    if trace and res.exec_time_ns is not None:
        print(f"HW exec time: {res.exec_time_ns} ns")

    out = np.concatenate([res.results[i]["out"] for i in range(N_CORES)], axis=0)
    return out
